# revision 1
# baseline (speedup 1.0000x reference)
"""Trainium2 Bass kernel for nn_EnsembleModel (histogram binning + gather-blend).

Math (reference):
    key[i,p1,p2]   = adds[i,p1]*T + adds[i,p2]
    tab_arc[k]     = segment_sum(a_arc.flat, key)           # [T^2]
    tab_rel[k,r]   = segment_sum(a_rel.flat(-1,R), key)     # [T^2, R]
    out_arc        = s_arc + tab_arc[pos-pair-key] * ALPHA
    out_rel        = s_rel + tab_rel[pos-pair-key] * ALPHA

Strategy: data-parallel over the 16 buckets (2 per core, 8 cores).
Histogram and gather are expressed as one-hot matmuls on the TensorEngine
(fp32, exact): with O = onehot(adds) [S,T] and Q = onehot(pos) [S,T],
    tab  = sum_i O_i^T A_i O_i          (bilinear segment-sum)
    gath = Q_i tab Q_i^T                (bilinear gather)
The tiny [T,T*(R+1)+...] tables are AllReduced across the 8 cores between
the two phases. One-hot matrices are built host-side from the int index
tensors (index preprocessing only; all float math runs on device).
"""

import numpy as np

import concourse.bacc as bacc
import concourse.tile as tile
from concourse import mybir
from concourse import bass_utils

F32 = mybir.dt.float32

# Problem shapes (hardcoded per contract).
B, S, R, T = 16, 160, 40, 50
ALPHA = 0.3
N_CORES = 8
BPC = B // N_CORES          # buckets per core = 2
PR = S * R                  # 6400  (p2, r) flat
TR = T * R                  # 2000  (t1, r) flat
P_LO, P_HI = 128, S - 128   # position-dim split across partitions
W_CH = 512                  # matmul moving-operand chunk (fp32 max)


def _chunks(total, w=W_CH):
    return [(s, min(w, total - s)) for s in range(0, total, w)]


def _build():
    nc = bacc.Bacc("TRN2", target_bir_lowering=False, debug=False,
                   num_devices=N_CORES)

    a_arc = nc.dram_tensor("a_arc", [BPC, S, S], F32, kind="ExternalInput")
    a_rel = nc.dram_tensor("a_rel", [BPC, S, S, R], F32, kind="ExternalInput")
    s_arc = nc.dram_tensor("s_arc", [BPC, S, S], F32, kind="ExternalInput")
    s_rel = nc.dram_tensor("s_rel", [BPC, S, S, R], F32, kind="ExternalInput")
    oh_adds = nc.dram_tensor("oh_adds", [BPC, S, T], F32, kind="ExternalInput")
    ohT_pos = nc.dram_tensor("ohT_pos", [BPC, T, S], F32, kind="ExternalInput")
    out_arc = nc.dram_tensor("out_arc", [BPC, S, S], F32, kind="ExternalOutput")
    out_rel = nc.dram_tensor("out_rel", [BPC, S, S, R], F32, kind="ExternalOutput")

    TAB_W = TR + T          # 2050: rel table cols 0:2000, arc table cols 2000:2050

    with tile.TileContext(nc) as tc:
        with (
            tc.tile_pool(name="consts", bufs=1) as consts,
            tc.tile_pool(name="work", bufs=1) as work,
            tc.tile_pool(name="achunk", bufs=4) as achunk,
            tc.tile_pool(name="schunk", bufs=4) as schunk,
            tc.tile_pool(name="obuf", bufs=1) as obuf,
            tc.tile_pool(name="dram", bufs=1, space="DRAM") as dram,
        ):
            # ---- constants: one-hot matrices for both buckets ----
            O_lo, O_hi, QT = [], [], []
            for i in range(BPC):
                olo = consts.tile([P_LO, T], F32, tag=f"olo{i}")
                ohi = consts.tile([P_HI, T], F32, tag=f"ohi{i}")
                qt = consts.tile([T, S], F32, tag=f"qt{i}")
                nc.sync.dma_start(out=olo[:], in_=oh_adds[i, 0:P_LO])
                nc.sync.dma_start(out=ohi[:], in_=oh_adds[i, P_LO:S])
                nc.sync.dma_start(out=qt[:], in_=ohT_pos[i])
                O_lo.append(olo)
                O_hi.append(ohi)
                QT.append(qt)

            # =========== Phase 1: local histogram into PSUM tables ===========
            with (
                tc.tile_pool(name="ps_work", bufs=2, space="PSUM") as ps_work,
                tc.tile_pool(name="ps_tab", bufs=1, space="PSUM") as ps_tab,
            ):
                tab_ps = ps_tab.tile([T, TR], F32, tag="tab")        # 4 banks
                taba_ps = ps_tab.tile([T, T], F32, tag="taba")       # 1 bank

                for i in range(BPC):
                    arel_i = a_rel[i].rearrange("a b c -> a (b c)")  # [160, 6400]
                    U = work.tile([T, PR], F32, tag="U")
                    # h1: U[t1,(p2 r)] = sum_p1 O[p1,t1] * A[p1,(p2 r)]
                    for c0, w in _chunks(PR):
                        alo = achunk.tile([P_LO, W_CH], F32, tag="alo")
                        ahi = achunk.tile([P_HI, W_CH], F32, tag="ahi")
                        nc.sync.dma_start(out=alo[:, :w], in_=arel_i[0:P_LO, c0:c0 + w])
                        nc.sync.dma_start(out=ahi[:, :w], in_=arel_i[P_LO:S, c0:c0 + w])
                        psu = ps_work.tile([T, W_CH], F32, tag="mm")
                        nc.tensor.matmul(psu[:, :w], lhsT=O_lo[i][:], rhs=alo[:, :w],
                                         start=True, stop=False)
                        nc.tensor.matmul(psu[:, :w], lhsT=O_hi[i][:], rhs=ahi[:, :w],
                                         start=False, stop=True)
                        nc.scalar.copy(out=U[:, c0:c0 + w], in_=psu[:, :w])
                    # permute U[t1,(p2 r)] -> Up[p2,(t1 r)]  (SBUF->SBUF DMA per t1)
                    up_lo = work.tile([P_LO, TR], F32, tag="uplo")
                    up_hi = work.tile([P_HI, TR], F32, tag="uphi")
                    for t1 in range(T):
                        nc.gpsimd.dma_start(out=up_lo[:, t1 * R:(t1 + 1) * R],
                                            in_=U[t1:t1 + 1, 0:P_LO * R])
                        nc.gpsimd.dma_start(out=up_hi[:, t1 * R:(t1 + 1) * R],
                                            in_=U[t1:t1 + 1, P_LO * R:PR])
                    # h2: tabT[t2,(t1 r)] += sum_p2 O[p2,t2] * Up[p2,(t1 r)]
                    for ci, (c0, w) in enumerate(_chunks(TR)):
                        nc.tensor.matmul(tab_ps[:, c0:c0 + w], lhsT=O_lo[i][:],
                                         rhs=up_lo[:, c0:c0 + w],
                                         start=(i == 0), stop=False)
                        nc.tensor.matmul(tab_ps[:, c0:c0 + w], lhsT=O_hi[i][:],
                                         rhs=up_hi[:, c0:c0 + w],
                                         start=False, stop=(i == BPC - 1))
                    # arc: UTa[p2,t1] = sum_p1 Aarc[p1,p2]*O[p1,t1]  (Aarc stationary)
                    aarc_lo = work.tile([P_LO, S], F32, tag="aarclo")
                    aarc_hi = work.tile([P_HI, S], F32, tag="aarchi")
                    nc.sync.dma_start(out=aarc_lo[:], in_=a_arc[i, 0:P_LO])
                    nc.sync.dma_start(out=aarc_hi[:], in_=a_arc[i, P_LO:S])
                    uta_lo = work.tile([P_LO, T], F32, tag="utalo")
                    uta_hi = work.tile([P_HI, T], F32, tag="utahi")
                    for mlo, mhi, m0, m1 in ((uta_lo, None, 0, P_LO),
                                             (None, uta_hi, P_LO, S)):
                        dst = mlo if mlo is not None else uta_hi
                        psa = ps_work.tile([P_LO, T], F32, tag="mm")
                        nc.tensor.matmul(psa[:m1 - m0, :], lhsT=aarc_lo[:, m0:m1],
                                         rhs=O_lo[i][:], start=True, stop=False)
                        nc.tensor.matmul(psa[:m1 - m0, :], lhsT=aarc_hi[:, m0:m1],
                                         rhs=O_hi[i][:], start=False, stop=True)
                        nc.vector.tensor_copy(out=dst[:], in_=psa[:m1 - m0, :])
                    # arc h2: taba[t2,t1] += sum_p2 O[p2,t2] * UTa[p2,t1]
                    nc.tensor.matmul(taba_ps[:], lhsT=O_lo[i][:], rhs=uta_lo[:],
                                     start=(i == 0), stop=False)
                    nc.tensor.matmul(taba_ps[:], lhsT=O_hi[i][:], rhs=uta_hi[:],
                                     start=False, stop=(i == BPC - 1))

                # evacuate tables to SBUF, then DRAM for the collective
                ccin = work.tile([T, TAB_W], F32, tag="ccin")
                nc.vector.tensor_copy(out=ccin[:, 0:TR], in_=tab_ps[:])
                nc.vector.tensor_copy(out=ccin[:, TR:TAB_W], in_=taba_ps[:])

            cc_in = dram.tile([T, TAB_W], F32, tag="ccin_d")
            cc_out = dram.tile([T, TAB_W], F32, tag="ccout_d")
            nc.sync.dma_start(out=cc_in[:], in_=ccin[:])
            nc.gpsimd.collective_compute(
                "AllReduce",
                mybir.AluOpType.add,
                replica_groups=[list(range(N_CORES))],
                ins=[cc_in[:].opt()],
                outs=[cc_out[:].opt()],
            )
            # fetch reduced table, fold in ALPHA
            tabf = work.tile([T, TAB_W], F32, tag="tabf")
            tabs = work.tile([T, TAB_W], F32, tag="tabs")
            nc.sync.dma_start(out=tabf[:], in_=cc_out[:])
            nc.vector.tensor_scalar_mul(tabs[:], tabf[:], ALPHA)
            tab_rel_s = tabs[:, 0:TR]     # [t2, (t1 r)] * ALPHA
            tab_arc_s = tabs[:, TR:TAB_W]  # [t2, t1] * ALPHA

            # =========== Phase 2: gather + blend ===========
            with tc.tile_pool(name="ps_g", bufs=2, space="PSUM") as ps_g:
                for i in range(BPC):
                    # g1: W[p2,(t1 r)] = sum_t2 QT[t2,p2] * tabT[t2,(t1 r)]
                    w_lo = work.tile([P_LO, TR], F32, tag="wlo")
                    w_hi = work.tile([P_HI, TR], F32, tag="whi")
                    for (wt, ps0, ps1) in ((w_lo, 0, P_LO), (w_hi, P_LO, S)):
                        for c0, w in _chunks(TR):
                            psw = ps_g.tile([P_LO, W_CH], F32, tag="gmm")
                            nc.tensor.matmul(psw[:ps1 - ps0, :w],
                                             lhsT=QT[i][:, ps0:ps1],
                                             rhs=tab_rel_s[:, c0:c0 + w],
                                             start=True, stop=True)
                            nc.scalar.copy(out=wt[:, c0:c0 + w], in_=psw[:ps1 - ps0, :w])
                    # permute W[p2,(t1 r)] -> H[t1,(p2 r)]
                    H = work.tile([T, PR], F32, tag="H")
                    for t1 in range(T):
                        nc.gpsimd.dma_start(out=H[t1:t1 + 1, 0:P_LO * R],
                                            in_=w_lo[:, t1 * R:(t1 + 1) * R])
                        nc.gpsimd.dma_start(out=H[t1:t1 + 1, P_LO * R:PR],
                                            in_=w_hi[:, t1 * R:(t1 + 1) * R])
                    # arc: X[t1,p2] = sum_t2 taba[t2,t1] * QT[t2,p2]
                    xps = ps_g.tile([T, S], F32, tag="gsm")
                    nc.tensor.matmul(xps[:], lhsT=tab_arc_s, rhs=QT[i][:],
                                     start=True, stop=True)
                    X = work.tile([T, S], F32, tag="X")
                    nc.vector.tensor_copy(out=X[:], in_=xps[:])

                    # g2 + blend (rel): out[p1,(p2 r)] = s_rel + sum_t1 QT[t1,p1]*H
                    srel_i = s_rel[i].rearrange("a b c -> a (b c)")
                    orel_i = out_rel[i].rearrange("a b c -> a (b c)")
                    o_lo = obuf.tile([P_LO, PR], F32, tag="orlo")
                    o_hi = obuf.tile([P_HI, PR], F32, tag="orhi")
                    for (ot, ps0, ps1) in ((o_lo, 0, P_LO), (o_hi, P_LO, S)):
                        for c0, w in _chunks(PR):
                            sch = schunk.tile([P_LO, W_CH], F32, tag="sch")
                            nc.sync.dma_start(out=sch[:ps1 - ps0, :w],
                                              in_=srel_i[ps0:ps1, c0:c0 + w])
                            psg = ps_g.tile([P_LO, W_CH], F32, tag="gmm")
                            nc.tensor.matmul(psg[:ps1 - ps0, :w],
                                             lhsT=QT[i][:, ps0:ps1],
                                             rhs=H[:, c0:c0 + w],
                                             start=True, stop=True)
                            nc.vector.tensor_add(out=ot[:, c0:c0 + w],
                                                 in0=psg[:ps1 - ps0, :w],
                                                 in1=sch[:ps1 - ps0, :w])
                        nc.sync.dma_start(out=orel_i[ps0:ps1, :], in_=ot[:])
                    # g2 + blend (arc)
                    oa_lo = obuf.tile([P_LO, S], F32, tag="oalo")
                    oa_hi = obuf.tile([P_HI, S], F32, tag="oahi")
                    for (ot, ps0, ps1) in ((oa_lo, 0, P_LO), (oa_hi, P_LO, S)):
                        sca = schunk.tile([P_LO, S], F32, tag="sca")
                        nc.sync.dma_start(out=sca[:ps1 - ps0, :],
                                          in_=s_arc[i, ps0:ps1])
                        psga = ps_g.tile([P_LO, S], F32, tag="gsm")
                        nc.tensor.matmul(psga[:ps1 - ps0, :],
                                         lhsT=QT[i][:, ps0:ps1], rhs=X[:],
                                         start=True, stop=True)
                        nc.vector.tensor_add(out=ot[:], in0=psga[:ps1 - ps0, :],
                                             in1=sca[:ps1 - ps0, :])
                        nc.sync.dma_start(out=out_arc[i, ps0:ps1], in_=ot[:])

    nc.compile()
    return nc


_NC_CACHE = None


def _get_nc():
    global _NC_CACHE
    if _NC_CACHE is None:
        _NC_CACHE = _build()
    return _NC_CACHE


def _run(inputs, trace=False):
    a_arc = np.asarray(inputs["a_arc"], dtype=np.float32)
    a_rel = np.asarray(inputs["a_rel"], dtype=np.float32)
    s_arc = np.asarray(inputs["s_arc"], dtype=np.float32)
    s_rel = np.asarray(inputs["s_rel"], dtype=np.float32)
    adds = np.asarray(inputs["adds"]).astype(np.int64)
    pos = np.asarray(inputs["pos"]).astype(np.int64)

    eye = np.arange(T, dtype=np.int64)
    oh_adds = (adds[:, :, None] == eye[None, None, :]).astype(np.float32)  # [B,S,T]
    ohT_pos = (pos[:, None, :] == eye[None, :, None]).astype(np.float32)   # [B,T,S]

    nc = _get_nc()
    in_maps = []
    for c in range(N_CORES):
        sl = slice(c * BPC, (c + 1) * BPC)
        in_maps.append({
            "a_arc": np.ascontiguousarray(a_arc[sl]),
            "a_rel": np.ascontiguousarray(a_rel[sl]),
            "s_arc": np.ascontiguousarray(s_arc[sl]),
            "s_rel": np.ascontiguousarray(s_rel[sl]),
            "oh_adds": np.ascontiguousarray(oh_adds[sl]),
            "ohT_pos": np.ascontiguousarray(ohT_pos[sl]),
        })
    res = bass_utils.run_bass_kernel_spmd(
        nc, in_maps, core_ids=list(range(N_CORES)), trace=trace)
    out_arc = np.concatenate([res.results[c]["out_arc"] for c in range(N_CORES)], axis=0)
    out_rel = np.concatenate([res.results[c]["out_rel"] for c in range(N_CORES)], axis=0)
    return (out_arc, out_rel), res


def kernel(**inputs):
    outs, _ = _run(inputs, trace=False)
    return outs


if __name__ == "__main__":
    rng = np.random.default_rng(0)
    inputs = {
        "a_arc": rng.standard_normal((B, S, S), dtype=np.float32),
        "a_rel": rng.standard_normal((B, S, S, R), dtype=np.float32),
        "s_arc": rng.standard_normal((B, S, S), dtype=np.float32),
        "s_rel": rng.standard_normal((B, S, S, R), dtype=np.float32),
        "adds": rng.integers(0, T, size=(B, S)),
        "pos": rng.integers(0, T, size=(B, S)),
        "n_tags": T,
    }
    (oa, orr), _ = _run(inputs)
    # numpy reference
    key = (inputs["adds"][:, :, None] * T + inputs["adds"][:, None, :]).reshape(-1)
    tab_arc = np.zeros(T * T, np.float32)
    np.add.at(tab_arc, key, inputs["a_arc"].reshape(-1))
    tab_rel = np.zeros((T * T, R), np.float32)
    np.add.at(tab_rel, key, inputs["a_rel"].reshape(-1, R))
    kp = inputs["pos"][:, :, None] * T + inputs["pos"][:, None, :]
    ea = inputs["s_arc"] + tab_arc[kp] * ALPHA
    er = inputs["s_rel"] + tab_rel[kp] * ALPHA
    print("arc rel err:", np.linalg.norm(oa - ea) / np.linalg.norm(ea))
    print("rel rel err:", np.linalg.norm(orr - er) / np.linalg.norm(er))


# revision 3
# speedup vs baseline: 1.1157x; 1.1157x over previous
"""Trainium2 Bass kernel for nn_EnsembleModel (histogram binning + gather-blend).

Math (reference):
    key[i,p1,p2]   = adds[i,p1]*T + adds[i,p2]
    tab_arc[k]     = segment_sum(a_arc.flat, key)           # [T^2]
    tab_rel[k,r]   = segment_sum(a_rel.flat(-1,R), key)     # [T^2, R]
    out_arc        = s_arc + tab_arc[pos-pair-key] * ALPHA
    out_rel        = s_rel + tab_rel[pos-pair-key] * ALPHA

Strategy: data-parallel over the 16 buckets (2 per core, 8 cores).
Histogram and gather are expressed as one-hot matmuls on the TensorEngine
(fp32, exact): with O = onehot(adds) [S,T] and Q = onehot(pos) [S,T],
    tab  = sum_i O_i^T A_i O_i          (bilinear segment-sum)
    gath = Q_i tab Q_i^T                (bilinear gather)
The [T, T*R+T] tables are AllReduced across the 8 cores between phases.
The (t1,p2)-transposes between the two contractions of each bilinear form
go through small DRAM bounce buffers: contiguous store, strided read
(rearranged DRAM access pattern), which keeps the DMA instruction count
tiny. One-hot matrices are built host-side from the int index tensors
(index preprocessing only; all float math runs on device).
"""

import numpy as np

import concourse.bacc as bacc
import concourse.tile as tile
from concourse import mybir
from concourse import bass_utils

F32 = mybir.dt.float32

# Problem shapes (hardcoded per contract).
B, S, R, T = 16, 160, 40, 50
ALPHA = 0.3
N_CORES = 8
BPC = B // N_CORES          # buckets per core = 2
PR = S * R                  # 6400  (p2, r) flat
TR = T * R                  # 2000  (t1, r) flat
P_LO, P_HI = 128, S - 128   # position-dim split across partitions
W_CH = 512                  # matmul moving-operand chunk (fp32 max)
TAB_W = TR + T              # 2050: rel table cols 0:2000, arc cols 2000:2050


def _chunks(total, w=W_CH):
    return [(s, min(w, total - s)) for s in range(0, total, w)]


def _build():
    nc = bacc.Bacc("TRN2", target_bir_lowering=False, debug=False,
                   num_devices=N_CORES)

    a_arc = nc.dram_tensor("a_arc", [BPC, S, S], F32, kind="ExternalInput")
    a_rel = nc.dram_tensor("a_rel", [BPC, S, S, R], F32, kind="ExternalInput")
    s_arc = nc.dram_tensor("s_arc", [BPC, S, S], F32, kind="ExternalInput")
    s_rel = nc.dram_tensor("s_rel", [BPC, S, S, R], F32, kind="ExternalInput")
    oh_adds = nc.dram_tensor("oh_adds", [BPC, S, T], F32, kind="ExternalInput")
    ohT_pos = nc.dram_tensor("ohT_pos", [BPC, T, S], F32, kind="ExternalInput")
    out_arc = nc.dram_tensor("out_arc", [BPC, S, S], F32, kind="ExternalOutput")
    out_rel = nc.dram_tensor("out_rel", [BPC, S, S, R], F32, kind="ExternalOutput")

    with tile.TileContext(nc) as tc:
        with (
            tc.tile_pool(name="consts", bufs=1) as consts,
            tc.tile_pool(name="big", bufs=1) as big,
            tc.tile_pool(name="med", bufs=1) as med,
            tc.tile_pool(name="tabs", bufs=1) as tabp,
            tc.tile_pool(name="dram", bufs=1, space="DRAM") as dram,
        ):
            # ---- constants: one-hot matrices for both buckets ----
            O_lo, O_hi, QT = [], [], []
            for i in range(BPC):
                olo = consts.tile([P_LO, T], F32, tag=f"olo{i}")
                ohi = consts.tile([P_HI, T], F32, tag=f"ohi{i}")
                qt = consts.tile([T, S], F32, tag=f"qt{i}")
                nc.sync.dma_start(out=olo[:], in_=oh_adds[i, 0:P_LO])
                nc.sync.dma_start(out=ohi[:], in_=oh_adds[i, P_LO:S])
                nc.sync.dma_start(out=qt[:], in_=ohT_pos[i])
                O_lo.append(olo)
                O_hi.append(ohi)
                QT.append(qt)

            # DRAM bounce buffers for the (t1 <-> p2) permutes
            u_d = [dram.tile([T, PR], F32, tag=f"u_d{i}", name=f"u_d{i}") for i in range(BPC)]
            w_d = [dram.tile([S, TR], F32, tag=f"w_d{i}", name=f"w_d{i}") for i in range(BPC)]

            # =========== Phase 1: local histogram into PSUM tables ===========
            with (
                tc.tile_pool(name="ps_work", bufs=2, space="PSUM") as ps_work,
                tc.tile_pool(name="ps_tab", bufs=1, space="PSUM") as ps_tab,
            ):
                tab_ps = ps_tab.tile([T, TR], F32, tag="tab")        # 4 banks
                taba_ps = ps_tab.tile([T, T], F32, tag="taba")       # 1 bank

                for i in range(BPC):
                    arel_i = a_rel[i].rearrange("a b c -> a (b c)")  # [160, 6400]
                    a_lo = big.tile([P_LO, PR], F32, tag="big0")
                    a_hi = big.tile([P_HI, PR], F32, tag="big1")
                    nc.sync.dma_start(out=a_lo[:], in_=arel_i[0:P_LO])
                    nc.sync.dma_start(out=a_hi[:], in_=arel_i[P_LO:S])
                    U = big.tile([T, PR], F32, tag="big2")
                    # h1: U[t1,(p2 r)] = sum_p1 O[p1,t1] * A[p1,(p2 r)]
                    for c0, w in _chunks(PR):
                        psu = ps_work.tile([T, W_CH], F32, tag="mm")
                        nc.tensor.matmul(psu[:, :w], lhsT=O_lo[i][:],
                                         rhs=a_lo[:, c0:c0 + w],
                                         start=True, stop=False)
                        nc.tensor.matmul(psu[:, :w], lhsT=O_hi[i][:],
                                         rhs=a_hi[:, c0:c0 + w],
                                         start=False, stop=True)
                        nc.scalar.copy(out=U[:, c0:c0 + w], in_=psu[:, :w])
                    # permute U[t1,(p2 r)] -> Up[p2,(t1 r)] via DRAM bounce:
                    # contiguous store, strided (rearranged) read.
                    nc.scalar.dma_start(out=u_d[i][:], in_=U[:])
                    u_perm = u_d[i][:].rearrange("t (p r) -> p t r", r=R)
                    up_lo = med.tile([P_LO, TR], F32, tag="med0")
                    up_hi = med.tile([P_HI, TR], F32, tag="med1")
                    nc.scalar.dma_start(out=up_lo[:], in_=u_perm[0:P_LO])
                    nc.scalar.dma_start(out=up_hi[:], in_=u_perm[P_LO:S])
                    # h2: tabT[t2,(t1 r)] += sum_p2 O[p2,t2] * Up[p2,(t1 r)]
                    for c0, w in _chunks(TR):
                        nc.tensor.matmul(tab_ps[:, c0:c0 + w], lhsT=O_lo[i][:],
                                         rhs=up_lo[:, c0:c0 + w],
                                         start=(i == 0), stop=False)
                        nc.tensor.matmul(tab_ps[:, c0:c0 + w], lhsT=O_hi[i][:],
                                         rhs=up_hi[:, c0:c0 + w],
                                         start=False, stop=(i == BPC - 1))
                    # arc: UTa[p2,t1] = sum_p1 Aarc[p1,p2]*O[p1,t1] (Aarc stationary)
                    aarc_lo = med.tile([P_LO, S], F32, tag="aarclo")
                    aarc_hi = med.tile([P_HI, S], F32, tag="aarchi")
                    nc.sync.dma_start(out=aarc_lo[:], in_=a_arc[i, 0:P_LO])
                    nc.sync.dma_start(out=aarc_hi[:], in_=a_arc[i, P_LO:S])
                    uta_lo = med.tile([P_LO, T], F32, tag="utalo")
                    uta_hi = med.tile([P_HI, T], F32, tag="utahi")
                    for dst, m0, m1 in ((uta_lo, 0, P_LO), (uta_hi, P_LO, S)):
                        psa = ps_work.tile([P_LO, T], F32, tag="mm")
                        nc.tensor.matmul(psa[:m1 - m0, :], lhsT=aarc_lo[:, m0:m1],
                                         rhs=O_lo[i][:], start=True, stop=False)
                        nc.tensor.matmul(psa[:m1 - m0, :], lhsT=aarc_hi[:, m0:m1],
                                         rhs=O_hi[i][:], start=False, stop=True)
                        nc.vector.tensor_copy(out=dst[:], in_=psa[:m1 - m0, :])
                    # arc h2: taba[t2,t1] += sum_p2 O[p2,t2] * UTa[p2,t1]
                    nc.tensor.matmul(taba_ps[:], lhsT=O_lo[i][:], rhs=uta_lo[:],
                                     start=(i == 0), stop=False)
                    nc.tensor.matmul(taba_ps[:], lhsT=O_hi[i][:], rhs=uta_hi[:],
                                     start=False, stop=(i == BPC - 1))

                # evacuate tables to SBUF, then DRAM for the collective
                ccin = tabp.tile([T, TAB_W], F32, tag="ccin")
                nc.vector.tensor_copy(out=ccin[:, 0:TR], in_=tab_ps[:])
                nc.vector.tensor_copy(out=ccin[:, TR:TAB_W], in_=taba_ps[:])

            cc_in = dram.tile([T, TAB_W], F32, tag="ccin_d")
            cc_out = dram.tile([T, TAB_W], F32, tag="ccout_d")
            nc.sync.dma_start(out=cc_in[:], in_=ccin[:])
            nc.gpsimd.collective_compute(
                "AllReduce",
                mybir.AluOpType.add,
                replica_groups=[list(range(N_CORES))],
                ins=[cc_in[:].opt()],
                outs=[cc_out[:].opt()],
            )
            # fetch reduced table, fold in ALPHA
            tabf = tabp.tile([T, TAB_W], F32, tag="tabf")
            tabs = tabp.tile([T, TAB_W], F32, tag="tabs")
            nc.sync.dma_start(out=tabf[:], in_=cc_out[:])
            nc.vector.tensor_scalar_mul(tabs[:], tabf[:], ALPHA)
            tab_rel_s = tabs[:, 0:TR]      # [t2, (t1 r)] * ALPHA
            tab_arc_s = tabs[:, TR:TAB_W]  # [t2, t1] * ALPHA

            # =========== Phase 2: gather + blend ===========
            with tc.tile_pool(name="ps_g", bufs=3, space="PSUM") as ps_g:
                for i in range(BPC):
                    # g1: W[p2,(t1 r)] = sum_t2 QT[t2,p2] * tabT[t2,(t1 r)]
                    w_lo = med.tile([P_LO, TR], F32, tag="med0")
                    w_hi = med.tile([P_HI, TR], F32, tag="med1")
                    for (wt, ps0, ps1) in ((w_lo, 0, P_LO), (w_hi, P_LO, S)):
                        for c0, w in _chunks(TR):
                            psw = ps_g.tile([P_LO, W_CH], F32, tag="gmm")
                            nc.tensor.matmul(psw[:ps1 - ps0, :w],
                                             lhsT=QT[i][:, ps0:ps1],
                                             rhs=tab_rel_s[:, c0:c0 + w],
                                             start=True, stop=True)
                            nc.scalar.copy(out=wt[:, c0:c0 + w],
                                           in_=psw[:ps1 - ps0, :w])
                    # permute W[p2,(t1 r)] -> H[t1,(p2 r)] via DRAM bounce
                    nc.scalar.dma_start(out=w_d[i][0:P_LO], in_=w_lo[:])
                    nc.scalar.dma_start(out=w_d[i][P_LO:S], in_=w_hi[:])
                    H = big.tile([T, PR], F32, tag="big4")
                    nc.scalar.dma_start(
                        out=H[:], in_=w_d[i][:].rearrange("p (t r) -> t p r", r=R))
                    # arc: X[t1,p2] = sum_t2 taba[t2,t1] * QT[t2,p2]
                    xps = ps_g.tile([T, S], F32, tag="gsm")
                    nc.tensor.matmul(xps[:], lhsT=tab_arc_s, rhs=QT[i][:],
                                     start=True, stop=True)
                    X = med.tile([T, S], F32, tag="X")
                    nc.vector.tensor_copy(out=X[:], in_=xps[:])

                    # g2 + blend (rel): out[p1,(p2 r)] = s_rel + sum_t1 QT[t1,p1]*H
                    srel_i = s_rel[i].rearrange("a b c -> a (b c)")
                    orel_i = out_rel[i].rearrange("a b c -> a (b c)")
                    s_lo = big.tile([P_LO, PR], F32, tag="big0")
                    s_hi = big.tile([P_HI, PR], F32, tag="big1")
                    nc.sync.dma_start(out=s_lo[:], in_=srel_i[0:P_LO])
                    nc.sync.dma_start(out=s_hi[:], in_=srel_i[P_LO:S])
                    o_lo = big.tile([P_LO, PR], F32, tag="big2")
                    o_hi = big.tile([P_HI, PR], F32, tag="big3")
                    for (ot, st, ps0, ps1) in ((o_lo, s_lo, 0, P_LO),
                                               (o_hi, s_hi, P_LO, S)):
                        for c0, w in _chunks(PR):
                            psg = ps_g.tile([P_LO, W_CH], F32, tag="gmm")
                            nc.tensor.matmul(psg[:ps1 - ps0, :w],
                                             lhsT=QT[i][:, ps0:ps1],
                                             rhs=H[:, c0:c0 + w],
                                             start=True, stop=True)
                            nc.vector.tensor_add(out=ot[:, c0:c0 + w],
                                                 in0=psg[:ps1 - ps0, :w],
                                                 in1=st[:, c0:c0 + w])
                        nc.sync.dma_start(out=orel_i[ps0:ps1, :], in_=ot[:])
                    # g2 + blend (arc)
                    sa_lo = med.tile([P_LO, S], F32, tag="aarclo")
                    sa_hi = med.tile([P_HI, S], F32, tag="aarchi")
                    nc.sync.dma_start(out=sa_lo[:], in_=s_arc[i, 0:P_LO])
                    nc.sync.dma_start(out=sa_hi[:], in_=s_arc[i, P_LO:S])
                    oa_lo = med.tile([P_LO, S], F32, tag="oalo")
                    oa_hi = med.tile([P_HI, S], F32, tag="oahi")
                    for (ot, st, ps0, ps1) in ((oa_lo, sa_lo, 0, P_LO),
                                               (oa_hi, sa_hi, P_LO, S)):
                        psga = ps_g.tile([P_LO, S], F32, tag="gsm")
                        nc.tensor.matmul(psga[:ps1 - ps0, :],
                                         lhsT=QT[i][:, ps0:ps1], rhs=X[:],
                                         start=True, stop=True)
                        nc.vector.tensor_add(out=ot[:], in0=psga[:ps1 - ps0, :],
                                             in1=st[:])
                        nc.sync.dma_start(out=out_arc[i, ps0:ps1], in_=ot[:])

    nc.compile()
    return nc


_NC_CACHE = None


def _get_nc():
    global _NC_CACHE
    if _NC_CACHE is None:
        _NC_CACHE = _build()
    return _NC_CACHE


def _run(inputs, trace=False):
    a_arc = np.asarray(inputs["a_arc"], dtype=np.float32)
    a_rel = np.asarray(inputs["a_rel"], dtype=np.float32)
    s_arc = np.asarray(inputs["s_arc"], dtype=np.float32)
    s_rel = np.asarray(inputs["s_rel"], dtype=np.float32)
    adds = np.asarray(inputs["adds"]).astype(np.int64)
    pos = np.asarray(inputs["pos"]).astype(np.int64)

    eye = np.arange(T, dtype=np.int64)
    oh_adds = (adds[:, :, None] == eye[None, None, :]).astype(np.float32)  # [B,S,T]
    ohT_pos = (pos[:, None, :] == eye[None, :, None]).astype(np.float32)   # [B,T,S]

    nc = _get_nc()
    in_maps = []
    for c in range(N_CORES):
        sl = slice(c * BPC, (c + 1) * BPC)
        in_maps.append({
            "a_arc": np.ascontiguousarray(a_arc[sl]),
            "a_rel": np.ascontiguousarray(a_rel[sl]),
            "s_arc": np.ascontiguousarray(s_arc[sl]),
            "s_rel": np.ascontiguousarray(s_rel[sl]),
            "oh_adds": np.ascontiguousarray(oh_adds[sl]),
            "ohT_pos": np.ascontiguousarray(ohT_pos[sl]),
        })
    res = bass_utils.run_bass_kernel_spmd(
        nc, in_maps, core_ids=list(range(N_CORES)), trace=trace)
    out_arc = np.concatenate([res.results[c]["out_arc"] for c in range(N_CORES)], axis=0)
    out_rel = np.concatenate([res.results[c]["out_rel"] for c in range(N_CORES)], axis=0)
    return (out_arc, out_rel), res


def kernel(**inputs):
    outs, _ = _run(inputs, trace=False)
    return outs


if __name__ == "__main__":
    rng = np.random.default_rng(0)
    inputs = {
        "a_arc": rng.standard_normal((B, S, S), dtype=np.float32),
        "a_rel": rng.standard_normal((B, S, S, R), dtype=np.float32),
        "s_arc": rng.standard_normal((B, S, S), dtype=np.float32),
        "s_rel": rng.standard_normal((B, S, S, R), dtype=np.float32),
        "adds": rng.integers(0, T, size=(B, S)),
        "pos": rng.integers(0, T, size=(B, S)),
        "n_tags": T,
    }
    (oa, orr), _ = _run(inputs)
    key = (inputs["adds"][:, :, None] * T + inputs["adds"][:, None, :]).reshape(-1)
    tab_arc = np.zeros(T * T, np.float32)
    np.add.at(tab_arc, key, inputs["a_arc"].reshape(-1))
    tab_rel = np.zeros((T * T, R), np.float32)
    np.add.at(tab_rel, key, inputs["a_rel"].reshape(-1, R))
    kp = inputs["pos"][:, :, None] * T + inputs["pos"][:, None, :]
    ea = inputs["s_arc"] + tab_arc[kp] * ALPHA
    er = inputs["s_rel"] + tab_rel[kp] * ALPHA
    print("arc rel err:", np.linalg.norm(oa - ea) / np.linalg.norm(ea))
    print("rel rel err:", np.linalg.norm(orr - er) / np.linalg.norm(er))


# revision 4
# speedup vs baseline: 1.2774x; 1.1449x over previous
"""Trainium2 Bass kernel for nn_EnsembleModel (histogram binning + gather-blend).

Math (reference):
    key[i,p1,p2]   = adds[i,p1]*T + adds[i,p2]
    tab_arc[k]     = segment_sum(a_arc.flat, key)           # [T^2]
    tab_rel[k,r]   = segment_sum(a_rel.flat(-1,R), key)     # [T^2, R]
    out_arc        = s_arc + tab_arc[pos-pair-key] * ALPHA
    out_rel        = s_rel + tab_rel[pos-pair-key] * ALPHA

Strategy: data-parallel over the 16 buckets (2 per core, 8 cores).
Histogram and gather are expressed as one-hot matmuls on the TensorEngine
(fp32, exact): with O = onehot(adds) [S,T] and Q = onehot(pos) [S,T],
    tab  = sum_i O_i^T A_i O_i          (bilinear segment-sum)
    gath = Q_i tab Q_i^T                (bilinear gather)
The [T, T*R+T] tables are AllReduced across the 8 cores between phases.
The (t1,p2)-transposes between the two contractions of each bilinear form
go through small DRAM bounce buffers: contiguous store, strided read
(rearranged DRAM access pattern), which keeps the DMA instruction count
tiny. One-hot matrices are built host-side from the int index tensors
(index preprocessing only; all float math runs on device).
"""

import numpy as np

import concourse.bacc as bacc
import concourse.tile as tile
from concourse import mybir
from concourse import bass_utils

F32 = mybir.dt.float32

# Problem shapes (hardcoded per contract).
B, S, R, T = 16, 160, 40, 50
ALPHA = 0.3
N_CORES = 8
BPC = B // N_CORES          # buckets per core = 2
PR = S * R                  # 6400  (p2, r) flat
TR = T * R                  # 2000  (t1, r) flat
P_LO, P_HI = 128, S - 128   # position-dim split across partitions
W_CH = 512                  # matmul moving-operand chunk (fp32 max)
TAB_W = TR + T              # 2050: rel table cols 0:2000, arc cols 2000:2050


def _chunks(total, w=W_CH):
    return [(s, min(w, total - s)) for s in range(0, total, w)]


def _build():
    nc = bacc.Bacc("TRN2", target_bir_lowering=False, debug=False,
                   num_devices=N_CORES)

    a_arc = nc.dram_tensor("a_arc", [BPC, S, S], F32, kind="ExternalInput")
    a_rel = nc.dram_tensor("a_rel", [BPC, S, S, R], F32, kind="ExternalInput")
    s_arc = nc.dram_tensor("s_arc", [BPC, S, S], F32, kind="ExternalInput")
    s_rel = nc.dram_tensor("s_rel", [BPC, S, S, R], F32, kind="ExternalInput")
    oh_adds = nc.dram_tensor("oh_adds", [BPC, S, T], F32, kind="ExternalInput")
    ohT_pos = nc.dram_tensor("ohT_pos", [BPC, T, S], F32, kind="ExternalInput")
    out_arc = nc.dram_tensor("out_arc", [BPC, S, S], F32, kind="ExternalOutput")
    out_rel = nc.dram_tensor("out_rel", [BPC, S, S, R], F32, kind="ExternalOutput")

    with tile.TileContext(nc) as tc:
        with (
            tc.tile_pool(name="consts", bufs=1) as consts,
            tc.tile_pool(name="big", bufs=1) as big,
            tc.tile_pool(name="med", bufs=1) as med,
            tc.tile_pool(name="tabs", bufs=1) as tabp,
            tc.tile_pool(name="dram", bufs=1, space="DRAM") as dram,
        ):
            # ---- constants: one-hot matrices for both buckets ----
            O_lo, O_hi, QT = [], [], []
            for i in range(BPC):
                olo = consts.tile([P_LO, T], F32, tag=f"olo{i}")
                ohi = consts.tile([P_HI, T], F32, tag=f"ohi{i}")
                qt = consts.tile([T, S], F32, tag=f"qt{i}")
                nc.sync.dma_start(out=olo[:], in_=oh_adds[i, 0:P_LO])
                nc.sync.dma_start(out=ohi[:], in_=oh_adds[i, P_LO:S])
                nc.sync.dma_start(out=qt[:], in_=ohT_pos[i])
                O_lo.append(olo)
                O_hi.append(ohi)
                QT.append(qt)

            # DRAM bounce buffers for the (t1 <-> p2) permutes
            u_d = [dram.tile([T, PR], F32, tag=f"u_d{i}", name=f"u_d{i}") for i in range(BPC)]
            w_d = [dram.tile([S, TR], F32, tag=f"w_d{i}", name=f"w_d{i}") for i in range(BPC)]

            # =========== Phase 1: local histogram into PSUM tables ===========
            with (
                tc.tile_pool(name="ps_work", bufs=3, space="PSUM") as ps_work,
                tc.tile_pool(name="ps_tab", bufs=1, space="PSUM") as ps_tab,
            ):
                tab_ps = ps_tab.tile([T, TR], F32, tag="tab")        # 4 banks
                taba_ps = ps_tab.tile([T, T], F32, tag="taba")       # 1 bank

                for i in range(BPC):
                    arel_i = a_rel[i].rearrange("a b c -> a (b c)")  # [160, 6400]
                    a_lo = big.tile([P_LO, PR], F32, tag="big0", bufs=2)
                    a_hi = big.tile([P_HI, PR], F32, tag="big1", bufs=2)
                    nc.sync.dma_start(out=a_lo[:], in_=arel_i[0:P_LO])
                    nc.sync.dma_start(out=a_hi[:], in_=arel_i[P_LO:S])
                    U = big.tile([T, PR], F32, tag="big2")
                    # h1: U[t1,(p2 r)] = sum_p1 O[p1,t1] * A[p1,(p2 r)]
                    for c0, w in _chunks(PR):
                        psu = ps_work.tile([T, W_CH], F32, tag="mm")
                        nc.tensor.matmul(psu[:, :w], lhsT=O_lo[i][:],
                                         rhs=a_lo[:, c0:c0 + w],
                                         start=True, stop=False)
                        nc.tensor.matmul(psu[:, :w], lhsT=O_hi[i][:],
                                         rhs=a_hi[:, c0:c0 + w],
                                         start=False, stop=True)
                        nc.scalar.copy(out=U[:, c0:c0 + w], in_=psu[:, :w])
                    # permute U[t1,(p2 r)] -> Up[p2,(t1 r)] via DRAM bounce:
                    # contiguous store, strided (rearranged) read.
                    nc.scalar.dma_start(out=u_d[i][:], in_=U[:])
                    u_perm = u_d[i][:].rearrange("t (p r) -> p t r", r=R)
                    up_lo = med.tile([P_LO, TR], F32, tag="med0", bufs=2)
                    up_hi = med.tile([P_HI, TR], F32, tag="med1", bufs=2)
                    nc.scalar.dma_start(out=up_lo[:], in_=u_perm[0:P_LO])
                    nc.scalar.dma_start(out=up_hi[:], in_=u_perm[P_LO:S])
                    # h2: tabT[t2,(t1 r)] += sum_p2 O[p2,t2] * Up[p2,(t1 r)]
                    for c0, w in _chunks(TR):
                        nc.tensor.matmul(tab_ps[:, c0:c0 + w], lhsT=O_lo[i][:],
                                         rhs=up_lo[:, c0:c0 + w],
                                         start=(i == 0), stop=False)
                        nc.tensor.matmul(tab_ps[:, c0:c0 + w], lhsT=O_hi[i][:],
                                         rhs=up_hi[:, c0:c0 + w],
                                         start=False, stop=(i == BPC - 1))
                    # arc: UTa[p2,t1] = sum_p1 Aarc[p1,p2]*O[p1,t1] (Aarc stationary)
                    aarc_lo = med.tile([P_LO, S], F32, tag="aarclo", bufs=2)
                    aarc_hi = med.tile([P_HI, S], F32, tag="aarchi", bufs=2)
                    nc.sync.dma_start(out=aarc_lo[:], in_=a_arc[i, 0:P_LO])
                    nc.sync.dma_start(out=aarc_hi[:], in_=a_arc[i, P_LO:S])
                    uta_lo = med.tile([P_LO, T], F32, tag="utalo")
                    uta_hi = med.tile([P_HI, T], F32, tag="utahi")
                    for dst, m0, m1 in ((uta_lo, 0, P_LO), (uta_hi, P_LO, S)):
                        psa = ps_work.tile([P_LO, T], F32, tag="mm")
                        nc.tensor.matmul(psa[:m1 - m0, :], lhsT=aarc_lo[:, m0:m1],
                                         rhs=O_lo[i][:], start=True, stop=False)
                        nc.tensor.matmul(psa[:m1 - m0, :], lhsT=aarc_hi[:, m0:m1],
                                         rhs=O_hi[i][:], start=False, stop=True)
                        nc.vector.tensor_copy(out=dst[:], in_=psa[:m1 - m0, :])
                    # arc h2: taba[t2,t1] += sum_p2 O[p2,t2] * UTa[p2,t1]
                    nc.tensor.matmul(taba_ps[:], lhsT=O_lo[i][:], rhs=uta_lo[:],
                                     start=(i == 0), stop=False)
                    nc.tensor.matmul(taba_ps[:], lhsT=O_hi[i][:], rhs=uta_hi[:],
                                     start=False, stop=(i == BPC - 1))

                # evacuate tables to SBUF, then DRAM for the collective
                ccin = tabp.tile([T, TAB_W], F32, tag="ccin")
                nc.vector.tensor_copy(out=ccin[:, 0:TR], in_=tab_ps[:])
                nc.vector.tensor_copy(out=ccin[:, TR:TAB_W], in_=taba_ps[:])

            cc_in = dram.tile([T, TAB_W], F32, tag="ccin_d")
            cc_out = dram.tile([T, TAB_W], F32, tag="ccout_d")
            nc.sync.dma_start(out=cc_in[:], in_=ccin[:])
            nc.gpsimd.collective_compute(
                "AllReduce",
                mybir.AluOpType.add,
                replica_groups=[list(range(N_CORES))],
                ins=[cc_in[:].opt()],
                outs=[cc_out[:].opt()],
            )
            # fetch reduced table, fold in ALPHA
            tabs = tabp.tile([T, TAB_W], F32, tag="ccin")
            nc.sync.dma_start(out=tabs[:], in_=cc_out[:])
            nc.vector.tensor_scalar_mul(tabs[:], tabs[:], ALPHA)
            tab_rel_s = tabs[:, 0:TR]      # [t2, (t1 r)] * ALPHA
            tab_arc_s = tabs[:, TR:TAB_W]  # [t2, t1] * ALPHA

            # =========== Phase 2: gather + blend ===========
            with tc.tile_pool(name="ps_g", bufs=3, space="PSUM") as ps_g:
                for i in range(BPC):
                    # g1: W[p2,(t1 r)] = sum_t2 QT[t2,p2] * tabT[t2,(t1 r)]
                    w_lo = med.tile([P_LO, TR], F32, tag="med0", bufs=2)
                    w_hi = med.tile([P_HI, TR], F32, tag="med1", bufs=2)
                    for (wt, ps0, ps1) in ((w_lo, 0, P_LO), (w_hi, P_LO, S)):
                        for c0, w in _chunks(TR):
                            psw = ps_g.tile([P_LO, W_CH], F32, tag="gmm")
                            nc.tensor.matmul(psw[:ps1 - ps0, :w],
                                             lhsT=QT[i][:, ps0:ps1],
                                             rhs=tab_rel_s[:, c0:c0 + w],
                                             start=True, stop=True)
                            nc.scalar.copy(out=wt[:, c0:c0 + w],
                                           in_=psw[:ps1 - ps0, :w])
                    # permute W[p2,(t1 r)] -> H[t1,(p2 r)] via DRAM bounce
                    nc.scalar.dma_start(out=w_d[i][0:P_LO], in_=w_lo[:])
                    nc.scalar.dma_start(out=w_d[i][P_LO:S], in_=w_hi[:])
                    H = big.tile([T, PR], F32, tag="big4")
                    nc.scalar.dma_start(
                        out=H[:], in_=w_d[i][:].rearrange("p (t r) -> t p r", r=R))
                    # arc: X[t1,p2] = sum_t2 taba[t2,t1] * QT[t2,p2]
                    xps = ps_g.tile([T, S], F32, tag="gsm")
                    nc.tensor.matmul(xps[:], lhsT=tab_arc_s, rhs=QT[i][:],
                                     start=True, stop=True)
                    X = med.tile([T, S], F32, tag="X")
                    nc.vector.tensor_copy(out=X[:], in_=xps[:])

                    # g2 + blend (rel): out[p1,(p2 r)] = s_rel + sum_t1 QT[t1,p1]*H
                    srel_i = s_rel[i].rearrange("a b c -> a (b c)")
                    orel_i = out_rel[i].rearrange("a b c -> a (b c)")
                    s_lo = big.tile([P_LO, PR], F32, tag="big0", bufs=2)
                    s_hi = big.tile([P_HI, PR], F32, tag="big1", bufs=2)
                    nc.sync.dma_start(out=s_lo[:], in_=srel_i[0:P_LO])
                    nc.sync.dma_start(out=s_hi[:], in_=srel_i[P_LO:S])
                    for (st, ps0, ps1) in ((s_lo, 0, P_LO), (s_hi, P_LO, S)):
                        for c0, w in _chunks(PR):
                            psg = ps_g.tile([P_LO, W_CH], F32, tag="gmm")
                            nc.tensor.matmul(psg[:ps1 - ps0, :w],
                                             lhsT=QT[i][:, ps0:ps1],
                                             rhs=H[:, c0:c0 + w],
                                             start=True, stop=True)
                            nc.vector.tensor_add(out=st[:, c0:c0 + w],
                                                 in0=psg[:ps1 - ps0, :w],
                                                 in1=st[:, c0:c0 + w])
                        nc.sync.dma_start(out=orel_i[ps0:ps1, :], in_=st[:])
                    # g2 + blend (arc)
                    sa_lo = med.tile([P_LO, S], F32, tag="aarclo", bufs=2)
                    sa_hi = med.tile([P_HI, S], F32, tag="aarchi", bufs=2)
                    nc.sync.dma_start(out=sa_lo[:], in_=s_arc[i, 0:P_LO])
                    nc.sync.dma_start(out=sa_hi[:], in_=s_arc[i, P_LO:S])
                    for (st, ps0, ps1) in ((sa_lo, 0, P_LO), (sa_hi, P_LO, S)):
                        psga = ps_g.tile([P_LO, S], F32, tag="gsm")
                        nc.tensor.matmul(psga[:ps1 - ps0, :],
                                         lhsT=QT[i][:, ps0:ps1], rhs=X[:],
                                         start=True, stop=True)
                        nc.vector.tensor_add(out=st[:], in0=psga[:ps1 - ps0, :],
                                             in1=st[:])
                        nc.sync.dma_start(out=out_arc[i, ps0:ps1], in_=st[:])

    nc.compile()
    return nc


_NC_CACHE = None


def _get_nc():
    global _NC_CACHE
    if _NC_CACHE is None:
        _NC_CACHE = _build()
    return _NC_CACHE


def _run(inputs, trace=False):
    a_arc = np.asarray(inputs["a_arc"], dtype=np.float32)
    a_rel = np.asarray(inputs["a_rel"], dtype=np.float32)
    s_arc = np.asarray(inputs["s_arc"], dtype=np.float32)
    s_rel = np.asarray(inputs["s_rel"], dtype=np.float32)
    adds = np.asarray(inputs["adds"]).astype(np.int64)
    pos = np.asarray(inputs["pos"]).astype(np.int64)

    eye = np.arange(T, dtype=np.int64)
    oh_adds = (adds[:, :, None] == eye[None, None, :]).astype(np.float32)  # [B,S,T]
    ohT_pos = (pos[:, None, :] == eye[None, :, None]).astype(np.float32)   # [B,T,S]

    nc = _get_nc()
    in_maps = []
    for c in range(N_CORES):
        sl = slice(c * BPC, (c + 1) * BPC)
        in_maps.append({
            "a_arc": np.ascontiguousarray(a_arc[sl]),
            "a_rel": np.ascontiguousarray(a_rel[sl]),
            "s_arc": np.ascontiguousarray(s_arc[sl]),
            "s_rel": np.ascontiguousarray(s_rel[sl]),
            "oh_adds": np.ascontiguousarray(oh_adds[sl]),
            "ohT_pos": np.ascontiguousarray(ohT_pos[sl]),
        })
    res = bass_utils.run_bass_kernel_spmd(
        nc, in_maps, core_ids=list(range(N_CORES)), trace=trace)
    out_arc = np.concatenate([res.results[c]["out_arc"] for c in range(N_CORES)], axis=0)
    out_rel = np.concatenate([res.results[c]["out_rel"] for c in range(N_CORES)], axis=0)
    return (out_arc, out_rel), res


def kernel(**inputs):
    outs, _ = _run(inputs, trace=False)
    return outs


if __name__ == "__main__":
    rng = np.random.default_rng(0)
    inputs = {
        "a_arc": rng.standard_normal((B, S, S), dtype=np.float32),
        "a_rel": rng.standard_normal((B, S, S, R), dtype=np.float32),
        "s_arc": rng.standard_normal((B, S, S), dtype=np.float32),
        "s_rel": rng.standard_normal((B, S, S, R), dtype=np.float32),
        "adds": rng.integers(0, T, size=(B, S)),
        "pos": rng.integers(0, T, size=(B, S)),
        "n_tags": T,
    }
    (oa, orr), _ = _run(inputs)
    key = (inputs["adds"][:, :, None] * T + inputs["adds"][:, None, :]).reshape(-1)
    tab_arc = np.zeros(T * T, np.float32)
    np.add.at(tab_arc, key, inputs["a_arc"].reshape(-1))
    tab_rel = np.zeros((T * T, R), np.float32)
    np.add.at(tab_rel, key, inputs["a_rel"].reshape(-1, R))
    kp = inputs["pos"][:, :, None] * T + inputs["pos"][:, None, :]
    ea = inputs["s_arc"] + tab_arc[kp] * ALPHA
    er = inputs["s_rel"] + tab_rel[kp] * ALPHA
    print("arc rel err:", np.linalg.norm(oa - ea) / np.linalg.norm(ea))
    print("rel rel err:", np.linalg.norm(orr - er) / np.linalg.norm(er))


# revision 5
# speedup vs baseline: 1.7283x; 1.3530x over previous
"""Trainium2 Bass kernel for nn_EnsembleModel (histogram binning + gather-blend).

Math (reference):
    key[i,p1,p2]   = adds[i,p1]*T + adds[i,p2]
    tab_arc[k]     = segment_sum(a_arc.flat, key)           # [T^2]
    tab_rel[k,r]   = segment_sum(a_rel.flat(-1,R), key)     # [T^2, R]
    out_arc        = s_arc + tab_arc[pos-pair-key] * ALPHA
    out_rel        = s_rel + tab_rel[pos-pair-key] * ALPHA

Strategy: data-parallel over the 16 buckets (2 per core, 8 cores).
Histogram and gather are expressed as one-hot matmuls on the TensorEngine
(fp32, exact): with O = onehot(adds) [S,T] and Q = onehot(pos) [S,T],
    tab  = sum_i O_i^T A_i O_i          (bilinear segment-sum)
    gath = Q_i tab Q_i^T                (bilinear gather)
The [T, T*R+T] tables are AllReduced across the 8 cores between phases.
The (t1,p2)-transposes between the two contractions of each bilinear form
go through small DRAM bounce buffers: contiguous store, strided read
(rearranged DRAM access pattern), which keeps the DMA instruction count
tiny. One-hot matrices are built host-side from the int index tensors
(index preprocessing only; all float math runs on device).
"""

import numpy as np

import concourse.bass as bass
import concourse.bacc as bacc
import concourse.tile as tile
from concourse import mybir
from concourse import bass_utils

F32 = mybir.dt.float32

# Problem shapes (hardcoded per contract).
B, S, R, T = 16, 160, 40, 50
ALPHA = 0.3
N_CORES = 8
BPC = B // N_CORES          # buckets per core = 2
PR = S * R                  # 6400  (p2, r) flat
TR = T * R                  # 2000  (t1, r) flat
P_LO, P_HI = 128, S - 128   # position-dim split across partitions
W_CH = 512                  # matmul moving-operand chunk (fp32 max)
TAB_W = TR + T              # 2050: rel table cols 0:2000, arc cols 2000:2050


def _chunks(total, w=W_CH):
    return [(s, min(w, total - s)) for s in range(0, total, w)]


def _build():
    nc = bacc.Bacc("TRN2", target_bir_lowering=False, debug=False,
                   num_devices=N_CORES)

    a_arc = nc.dram_tensor("a_arc", [BPC, S, S], F32, kind="ExternalInput")
    a_rel = nc.dram_tensor("a_rel", [BPC, S, S, R], F32, kind="ExternalInput")
    s_arc = nc.dram_tensor("s_arc", [BPC, S, S], F32, kind="ExternalInput")
    s_rel = nc.dram_tensor("s_rel", [BPC, S, S, R], F32, kind="ExternalInput")
    oh_adds = nc.dram_tensor("oh_adds", [BPC, S, T], F32, kind="ExternalInput")
    ohT_pos = nc.dram_tensor("ohT_pos", [BPC, T, S], F32, kind="ExternalInput")
    pos_i32 = nc.dram_tensor("pos_i32", [BPC, S], mybir.dt.int32, kind="ExternalInput")
    out_arc = nc.dram_tensor("out_arc", [BPC, S, S], F32, kind="ExternalOutput")
    out_rel = nc.dram_tensor("out_rel", [BPC, S, S, R], F32, kind="ExternalOutput")

    with tile.TileContext(nc) as tc:
        with (
            tc.tile_pool(name="consts", bufs=1) as consts,
            tc.tile_pool(name="big", bufs=1) as big,
            tc.tile_pool(name="med", bufs=1) as med,
            tc.tile_pool(name="tabs", bufs=1) as tabp,
            tc.tile_pool(name="dram", bufs=1, space="DRAM") as dram,
        ):
            # ---- constants: one-hot matrices for both buckets ----
            O_lo, O_hi, QT = [], [], []
            for i in range(BPC):
                olo = consts.tile([P_LO, T], F32, tag=f"olo{i}")
                ohi = consts.tile([P_HI, T], F32, tag=f"ohi{i}")
                qt = consts.tile([T, S], F32, tag=f"qt{i}")
                nc.sync.dma_start(out=olo[:], in_=oh_adds[i, 0:P_LO])
                nc.sync.dma_start(out=ohi[:], in_=oh_adds[i, P_LO:S])
                nc.sync.dma_start(out=qt[:], in_=ohT_pos[i])
                O_lo.append(olo)
                O_hi.append(ohi)
                QT.append(qt)

            # DRAM bounce buffers for the (t1 <-> p2) permutes
            u_d = [dram.tile([T, PR], F32, tag=f"u_d{i}", name=f"u_d{i}") for i in range(BPC)]
            h_d = [dram.tile([T, PR], F32, tag=f"h_d{i}", name=f"h_d{i}") for i in range(BPC)]
            tabrel_d = dram.tile([T, TR], F32, tag="tabrel_d", name="tabrel_d")

            # =========== Phase 1: local histogram into PSUM tables ===========
            with (
                tc.tile_pool(name="ps_work", bufs=3, space="PSUM") as ps_work,
                tc.tile_pool(name="ps_tab", bufs=1, space="PSUM") as ps_tab,
            ):
                tab_ps = ps_tab.tile([T, TR], F32, tag="tab")        # 4 banks
                taba_ps = ps_tab.tile([T, T], F32, tag="taba")       # 1 bank

                for i in range(BPC):
                    arel_i = a_rel[i].rearrange("a b c -> a (b c)")  # [160, 6400]
                    a_lo = big.tile([P_LO, PR], F32, tag="big0", bufs=2)
                    a_hi = big.tile([P_HI, PR], F32, tag="big1", bufs=2)
                    nc.sync.dma_start(out=a_lo[:], in_=arel_i[0:P_LO])
                    nc.sync.dma_start(out=a_hi[:], in_=arel_i[P_LO:S])
                    U = big.tile([T, PR], F32, tag="big2")
                    # h1: U[t1,(p2 r)] = sum_p1 O[p1,t1] * A[p1,(p2 r)]
                    for ci, (c0, w) in enumerate(_chunks(PR)):
                        psu = ps_work.tile([T, W_CH], F32, tag="mm")
                        nc.tensor.matmul(psu[:, :w], lhsT=O_lo[i][:],
                                         rhs=a_lo[:, c0:c0 + w],
                                         start=True, stop=False)
                        nc.tensor.matmul(psu[:, :w], lhsT=O_hi[i][:],
                                         rhs=a_hi[:, c0:c0 + w],
                                         start=False, stop=True)
                        if ci % 2 == 0:
                            nc.scalar.copy(out=U[:, c0:c0 + w], in_=psu[:, :w])
                        else:
                            nc.vector.tensor_copy(out=U[:, c0:c0 + w], in_=psu[:, :w])
                    # permute U[t1,(p2 r)] -> Up[p2,(t1 r)] via DRAM bounce:
                    # contiguous store, strided (rearranged) read.
                    nc.scalar.dma_start(out=u_d[i][:], in_=U[:])
                    u_perm = u_d[i][:].rearrange("t (p r) -> p t r", r=R)
                    up_lo = med.tile([P_LO, TR], F32, tag="med0", bufs=2)
                    up_hi = med.tile([P_HI, TR], F32, tag="med1", bufs=2)
                    nc.scalar.dma_start(out=up_lo[:], in_=u_perm[0:P_LO])
                    nc.scalar.dma_start(out=up_hi[:], in_=u_perm[P_LO:S])
                    # h2: tabT[t2,(t1 r)] += sum_p2 O[p2,t2] * Up[p2,(t1 r)]
                    for c0, w in _chunks(TR):
                        nc.tensor.matmul(tab_ps[:, c0:c0 + w], lhsT=O_lo[i][:],
                                         rhs=up_lo[:, c0:c0 + w],
                                         start=(i == 0), stop=False)
                        nc.tensor.matmul(tab_ps[:, c0:c0 + w], lhsT=O_hi[i][:],
                                         rhs=up_hi[:, c0:c0 + w],
                                         start=False, stop=(i == BPC - 1))
                    # arc: UTa[p2,t1] = sum_p1 Aarc[p1,p2]*O[p1,t1] (Aarc stationary)
                    aarc_lo = med.tile([P_LO, S], F32, tag="aarclo", bufs=2)
                    aarc_hi = med.tile([P_HI, S], F32, tag="aarchi", bufs=2)
                    nc.sync.dma_start(out=aarc_lo[:], in_=a_arc[i, 0:P_LO])
                    nc.sync.dma_start(out=aarc_hi[:], in_=a_arc[i, P_LO:S])
                    uta_lo = med.tile([P_LO, T], F32, tag="utalo")
                    uta_hi = med.tile([P_HI, T], F32, tag="utahi")
                    for dst, m0, m1 in ((uta_lo, 0, P_LO), (uta_hi, P_LO, S)):
                        psa = ps_work.tile([P_LO, T], F32, tag="mm")
                        nc.tensor.matmul(psa[:m1 - m0, :], lhsT=aarc_lo[:, m0:m1],
                                         rhs=O_lo[i][:], start=True, stop=False)
                        nc.tensor.matmul(psa[:m1 - m0, :], lhsT=aarc_hi[:, m0:m1],
                                         rhs=O_hi[i][:], start=False, stop=True)
                        nc.vector.tensor_copy(out=dst[:], in_=psa[:m1 - m0, :])
                    # arc h2: taba[t2,t1] += sum_p2 O[p2,t2] * UTa[p2,t1]
                    nc.tensor.matmul(taba_ps[:], lhsT=O_lo[i][:], rhs=uta_lo[:],
                                     start=(i == 0), stop=False)
                    nc.tensor.matmul(taba_ps[:], lhsT=O_hi[i][:], rhs=uta_hi[:],
                                     start=False, stop=(i == BPC - 1))

                # evacuate tables to SBUF, then DRAM for the collective
                ccin = tabp.tile([T, TAB_W], F32, tag="ccin")
                nc.vector.tensor_copy(out=ccin[:, 0:TR], in_=tab_ps[:])
                nc.vector.tensor_copy(out=ccin[:, TR:TAB_W], in_=taba_ps[:])

            cc_in = dram.tile([T, TAB_W], F32, tag="ccin_d")
            cc_out = dram.tile([T, TAB_W], F32, tag="ccout_d")
            nc.sync.dma_start(out=cc_in[:], in_=ccin[:])
            nc.gpsimd.collective_compute(
                "AllReduce",
                mybir.AluOpType.add,
                replica_groups=[list(range(N_CORES))],
                ins=[cc_in[:].opt()],
                outs=[cc_out[:].opt()],
            )
            # fetch reduced table, fold in ALPHA
            tabs = tabp.tile([T, TAB_W], F32, tag="ccin")
            nc.sync.dma_start(out=tabs[:], in_=cc_out[:])
            nc.vector.tensor_scalar_mul(tabs[:], tabs[:], ALPHA)
            tab_arc_s = tabs[:, TR:TAB_W]  # [t2, t1] * ALPHA
            nc.sync.dma_start(out=tabrel_d[:], in_=tabs[:, 0:TR])

            # =========== Phase 2: gather + blend ===========
            with tc.tile_pool(name="ps_g", bufs=3, space="PSUM") as ps_g:
                for i in range(BPC):
                    # pos index columns for the row-gathers
                    pc_lo = consts.tile([P_LO, 1], mybir.dt.int32, tag=f"pclo{i}")
                    pc_hi = consts.tile([P_HI, 1], mybir.dt.int32, tag=f"pchi{i}")
                    nc.sync.dma_start(out=pc_lo[:], in_=pos_i32[i, 0:P_LO])
                    nc.sync.dma_start(out=pc_hi[:], in_=pos_i32[i, P_LO:S])
                    # g1 as row-gather: W[p2,(t1 r)] = tabT_scaled[pos[p2]]
                    w_lo = med.tile([P_LO, TR], F32, tag="med0", bufs=2)
                    w_hi = med.tile([P_HI, TR], F32, tag="med1", bufs=2)
                    nc.gpsimd.indirect_dma_start(
                        out=w_lo[:], out_offset=None, in_=tabrel_d[:, :],
                        in_offset=bass.IndirectOffsetOnAxis(ap=pc_lo[:], axis=0))
                    nc.gpsimd.indirect_dma_start(
                        out=w_hi[:], out_offset=None, in_=tabrel_d[:, :],
                        in_offset=bass.IndirectOffsetOnAxis(ap=pc_hi[:], axis=0))
                    # permuted store: h_d[t1,(p2 r)] = W[p2,(t1 r)]
                    hperm = h_d[i][:].rearrange("t (p r) -> p t r", r=R)
                    nc.scalar.dma_start(out=hperm[0:P_LO], in_=w_lo[:])
                    nc.scalar.dma_start(out=hperm[P_LO:S], in_=w_hi[:])
                    # arc: X[t1,p2] = sum_t2 taba[t2,t1] * QT[t2,p2]
                    xps = ps_g.tile([T, S], F32, tag="gsm")
                    nc.tensor.matmul(xps[:], lhsT=tab_arc_s, rhs=QT[i][:],
                                     start=True, stop=True)
                    X = med.tile([T, S], F32, tag="X")
                    nc.vector.tensor_copy(out=X[:], in_=xps[:])

                    # g2 as row-gather: G[p1,(p2 r)] = h_d[pos[p1]]; blend on DVE
                    srel_i = s_rel[i].rearrange("a b c -> a (b c)")
                    orel_i = out_rel[i].rearrange("a b c -> a (b c)")
                    s_lo = big.tile([P_LO, PR], F32, tag="big0", bufs=2)
                    s_hi = big.tile([P_HI, PR], F32, tag="big1", bufs=2)
                    nc.sync.dma_start(out=s_lo[:], in_=srel_i[0:P_LO])
                    nc.sync.dma_start(out=s_hi[:], in_=srel_i[P_LO:S])
                    g_lo = big.tile([P_LO, PR], F32, tag="big2")
                    g_hi = big.tile([P_HI, PR], F32, tag="big3")
                    nc.gpsimd.indirect_dma_start(
                        out=g_lo[:], out_offset=None, in_=h_d[i][:, :],
                        in_offset=bass.IndirectOffsetOnAxis(ap=pc_lo[:], axis=0))
                    nc.gpsimd.indirect_dma_start(
                        out=g_hi[:], out_offset=None, in_=h_d[i][:, :],
                        in_offset=bass.IndirectOffsetOnAxis(ap=pc_hi[:], axis=0))
                    for (st, gt, ps0, ps1) in ((s_lo, g_lo, 0, P_LO),
                                               (s_hi, g_hi, P_LO, S)):
                        nc.vector.tensor_add(out=st[:], in0=gt[:], in1=st[:])
                        nc.sync.dma_start(out=orel_i[ps0:ps1, :], in_=st[:])
                    # g2 + blend (arc)
                    sa_lo = med.tile([P_LO, S], F32, tag="aarclo", bufs=2)
                    sa_hi = med.tile([P_HI, S], F32, tag="aarchi", bufs=2)
                    nc.sync.dma_start(out=sa_lo[:], in_=s_arc[i, 0:P_LO])
                    nc.sync.dma_start(out=sa_hi[:], in_=s_arc[i, P_LO:S])
                    for (st, ps0, ps1) in ((sa_lo, 0, P_LO), (sa_hi, P_LO, S)):
                        psga = ps_g.tile([P_LO, S], F32, tag="gsm")
                        nc.tensor.matmul(psga[:ps1 - ps0, :],
                                         lhsT=QT[i][:, ps0:ps1], rhs=X[:],
                                         start=True, stop=True)
                        nc.vector.tensor_add(out=st[:], in0=psga[:ps1 - ps0, :],
                                             in1=st[:])
                        nc.sync.dma_start(out=out_arc[i, ps0:ps1], in_=st[:])

    nc.compile()
    return nc


_NC_CACHE = None


def _get_nc():
    global _NC_CACHE
    if _NC_CACHE is None:
        _NC_CACHE = _build()
    return _NC_CACHE


def _run(inputs, trace=False):
    a_arc = np.asarray(inputs["a_arc"], dtype=np.float32)
    a_rel = np.asarray(inputs["a_rel"], dtype=np.float32)
    s_arc = np.asarray(inputs["s_arc"], dtype=np.float32)
    s_rel = np.asarray(inputs["s_rel"], dtype=np.float32)
    adds = np.asarray(inputs["adds"]).astype(np.int64)
    pos = np.asarray(inputs["pos"]).astype(np.int64)

    eye = np.arange(T, dtype=np.int64)
    oh_adds = (adds[:, :, None] == eye[None, None, :]).astype(np.float32)  # [B,S,T]
    ohT_pos = (pos[:, None, :] == eye[None, :, None]).astype(np.float32)   # [B,T,S]

    nc = _get_nc()
    in_maps = []
    for c in range(N_CORES):
        sl = slice(c * BPC, (c + 1) * BPC)
        in_maps.append({
            "a_arc": np.ascontiguousarray(a_arc[sl]),
            "a_rel": np.ascontiguousarray(a_rel[sl]),
            "s_arc": np.ascontiguousarray(s_arc[sl]),
            "s_rel": np.ascontiguousarray(s_rel[sl]),
            "oh_adds": np.ascontiguousarray(oh_adds[sl]),
            "ohT_pos": np.ascontiguousarray(ohT_pos[sl]),
            "pos_i32": np.ascontiguousarray(pos[sl].astype(np.int32)),
        })
    res = bass_utils.run_bass_kernel_spmd(
        nc, in_maps, core_ids=list(range(N_CORES)), trace=trace)
    out_arc = np.concatenate([res.results[c]["out_arc"] for c in range(N_CORES)], axis=0)
    out_rel = np.concatenate([res.results[c]["out_rel"] for c in range(N_CORES)], axis=0)
    return (out_arc, out_rel), res


def kernel(**inputs):
    outs, _ = _run(inputs, trace=False)
    return outs


if __name__ == "__main__":
    rng = np.random.default_rng(0)
    inputs = {
        "a_arc": rng.standard_normal((B, S, S), dtype=np.float32),
        "a_rel": rng.standard_normal((B, S, S, R), dtype=np.float32),
        "s_arc": rng.standard_normal((B, S, S), dtype=np.float32),
        "s_rel": rng.standard_normal((B, S, S, R), dtype=np.float32),
        "adds": rng.integers(0, T, size=(B, S)),
        "pos": rng.integers(0, T, size=(B, S)),
        "n_tags": T,
    }
    (oa, orr), _ = _run(inputs)
    key = (inputs["adds"][:, :, None] * T + inputs["adds"][:, None, :]).reshape(-1)
    tab_arc = np.zeros(T * T, np.float32)
    np.add.at(tab_arc, key, inputs["a_arc"].reshape(-1))
    tab_rel = np.zeros((T * T, R), np.float32)
    np.add.at(tab_rel, key, inputs["a_rel"].reshape(-1, R))
    kp = inputs["pos"][:, :, None] * T + inputs["pos"][:, None, :]
    ea = inputs["s_arc"] + tab_arc[kp] * ALPHA
    er = inputs["s_rel"] + tab_rel[kp] * ALPHA
    print("arc rel err:", np.linalg.norm(oa - ea) / np.linalg.norm(ea))
    print("rel rel err:", np.linalg.norm(orr - er) / np.linalg.norm(er))
